# revision 12
# baseline (speedup 1.0000x reference)
"""Level-1 3D Haar DWT on video [4,3,16,256,256] f32 -> 8 subbands
[4,3,8,128,128], pywt convention (cA=(x0+x1)/sqrt2, cD=(x0-x1)/sqrt2 over
frames, height, width).

Distribution: pure data parallel over the 8 frame pairs (F=16 -> 8
independent pairs); core k processes video[:, :, 2k:2k+2] with zero
cross-core communication.

Host side: inputs are cast to f16 (rel-err budget 2e-2 >> f16's ~5e-4
quantization) and laid out per core as x[v, f, rr, p, w] so every DMA
run is contiguous; outputs come back f16 and are upcast. This halves
HBM traffic on both sides -> ~17.6us memory roofline per core.

Per-core pipeline (Bass/Tile), ragged chunks of CH (b,c) pairs, v in
0..3 indexing 64-row blocks of each frame:
  load (sync HWDGE):  X[v] = x[v, f, rr, p0:p0+CH, :]  [128=(f,rr), CH*256]
  F+H stage (PE): stationary C[128,128] (+-2^-1.5, 4 nonzeros/col)
    pairs frames (f) and adjacent rows (rr=2j'+r) in ONE matmul pass:
    out j = t*64+q*32+j'. The whole 1/sqrt8 scale is folded into C so
    the W stage is a plain tensor_tensor (no custom-DVE uop table in
    the preamble). A few warmup matmuls run during the preamble dead
    zone to bring the PE out of its low p-state. (Folding the W stage
    into PE as parity-accumulating matmuls doubles PE columns and was
    measured slower - the PE never ramps to full clock here.)
  evac: PSUM pair-tile -> SBUF f16. v-pair 0: ACT copies odd w cols
    only and DVE combines even(PSUM)+odd(SBUF) at 1x. v-pair 1: ACT
    copies both parities so DVE runs all-SBUF f16 at 2x. This splits
    the evac+combine work evenly (~11.8us each).
  W stage (DVE): tensor_tensor add/sub -> YU f16.
  store (gpsimd SWDGE, freeing the scalar ring for ACT): per v-pair,
    y[j, v, e, p, w], 1KB runs.

Output DRAM y[j, v, e, p, w]: subband s = (t, q, e) with j = t*64+
q*32+j', h = 32v + j'.
"""

import math

import numpy as np

import concourse.bacc as bacc
import concourse.mybir as mybir
from concourse.bass_utils import run_bass_kernel_spmd
from concourse.tile import TileContext

F16 = mybir.dt.float16
F32 = mybir.dt.float32
NCORES = 8
NPAIRS = 12
CHUNKS = (2, 4, 4, 2)   # ragged: short first/last chunks trim fill/drain
CHMAX = max(CHUNKS)
C3 = (1.0 / math.sqrt(2.0)) ** 3
NWARM = 10

_CACHE = {}


def _cmat():
    """C[i, o]: i = f*64 + 2j'+r, o = t*64 + q*32 + j'; entry
    C3*sF(t,f)*sH(q,r) with a=(+,+), d=(+,-)."""
    c = np.zeros((128, 128), np.float16)
    for t in range(2):
        for q in range(2):
            for jp in range(32):
                o = t * 64 + q * 32 + jp
                for f in range(2):
                    sf = -1.0 if (t == 1 and f == 1) else 1.0
                    for r in range(2):
                        sh = -1.0 if (q == 1 and r == 1) else 1.0
                        c[f * 64 + 2 * jp + r, o] = np.float16(C3) * sf * sh
    return c


def _build_bass():
    nc = bacc.Bacc()
    x = nc.dram_tensor("x", [4, 2, 64, NPAIRS, 256], F16, kind="ExternalInput")
    cm = nc.dram_tensor("cmat", [128, 128], F16, kind="ExternalInput")
    y = nc.dram_tensor("y", [128, 4, 2, NPAIRS, 128], F16,
                       kind="ExternalOutput")

    add = mybir.AluOpType.add
    sub = mybir.AluOpType.subtract

    with TileContext(nc) as tc:
        with tc.tile_pool(name="const", bufs=1) as cpool, \
             tc.tile_pool(name="io", bufs=3) as io_pool, \
             tc.tile_pool(name="mid", bufs=3) as mid_pool, \
             tc.tile_pool(name="ps", bufs=1, space="PSUM") as ps_pool:
            Ct = cpool.tile([128, 128], F16, name="Ct")
            # on sync, ahead of the X loads: scalar's hoisted
            # ACT_TABLE_LOAD would delay Ct (and the PE warmup) by ~1.5us
            nc.sync.dma_start(out=Ct[:, :], in_=cm[:, :])
            # PE p-state warmup in the preamble shadow (results unused;
            # the first real matmul below overwrites with start=True)
            Pw = ps_pool.tile([128, 2, CHMAX * 256], F32, name="Pw", tag="Q0")
            for _ in range(NWARM):
                nc.tensor.matmul(Pw[:, 0, 0:128], Ct[:, :], Ct[:, :])
            p0 = 0
            for ci, CH in enumerate(CHUNKS):
                # one ~0.5MB load per v-pair: big DMAs keep the HWDGE ring
                # at line rate, pair granularity keeps the first-MM dep fine
                Xp, Pp = [], []
                for t in range(2):
                    Xt = io_pool.tile([128, 2, CH * 256], F16, name="X",
                                      tag=f"X{t}", bufs=3,
                                      padded_shape=[128, 2, CHMAX * 256])
                    nc.sync.dma_start(
                        out=Xt[:, :, :],
                        in_=x[2 * t:2 * t + 2, :, :, p0:p0 + CH, :]
                            .rearrange("v f rr p w -> (f rr) v (p w)"),
                    )
                    Xp.append(Xt)
                YU = io_pool.tile([128, 4, 2, CH * 128], F16, name="YU",
                                  tag="YU",
                                  padded_shape=[128, 4, 2, CHMAX * 128])
                # all 8 MMs of the chunk back-to-back: longer PE bursts
                # ride the p-state up; evacs overlap on ACT/DVE
                for t in range(2):      # v-pair tiles: v = 2t, 2t+1
                    P2 = ps_pool.tile([128, 2, CH * 256], F32, name="P",
                                      tag=f"Q{t}",
                                      padded_shape=[128, 2, CHMAX * 256])
                    for dv in range(2):
                        for n0 in range(0, CH * 256, 512):  # 1 PSUM bank/mm
                            n1 = min(n0 + 512, CH * 256)
                            nc.tensor.matmul(P2[:, dv, n0:n1], Ct[:, :],
                                             Xp[t][:, dv, n0:n1])
                    Pp.append(P2)
                for t in range(2):
                    P2 = Pp[t]
                    Ps = P2.rearrange("j u (pw r) -> j u pw r", r=2)
                    Pe, Po = Ps[:, :, :, 0], Ps[:, :, :, 1]
                    Od = mid_pool.tile([128, 2, CH * 128], F16, name="Od",
                                       tag=f"O{t}",
                                       padded_shape=[128, 2, CHMAX * 128])
                    nc.scalar.copy(Od[:, :, :], Po)
                    if t == 0:
                        # DVE reads evens straight from PSUM (1x)
                        ina, inb = Pe, Od[:, :, :]
                    else:
                        # ACT evacs evens too; DVE runs all-SBUF f16 (2x)
                        Ev = mid_pool.tile([128, 2, CH * 128], F16, name="Ev",
                                           tag="E1",
                                           padded_shape=[128, 2, CHMAX * 128])
                        nc.scalar.copy(Ev[:, :, :], Pe)
                        ina, inb = Ev[:, :, :], Od[:, :, :]
                    nc.vector.tensor_tensor(
                        YU[:, 2 * t:2 * t + 2, 0, :], ina, inb, add)
                    nc.vector.tensor_tensor(
                        YU[:, 2 * t:2 * t + 2, 1, :], ina, inb, sub)
                    nc.scalar.dma_start(
                        out=y[:, 2 * t:2 * t + 2, :, p0:p0 + CH, :]
                            .rearrange("j v e p w -> j v e (p w)"),
                        in_=YU[:, 2 * t:2 * t + 2, :, :],
                    )
                p0 += CH
    nc.compile()
    return nc


def _get_nc():
    if "nc" not in _CACHE:
        _CACHE["nc"] = _build_bass()
    return _CACHE["nc"]


def _shard_inputs(video):
    video = np.asarray(video, dtype=np.float16)
    cm = _cmat()
    in_maps = []
    for k in range(NCORES):
        sh = video[:, :, 2 * k:2 * k + 2]            # [4,3,2,256,256]
        sh = sh.reshape(NPAIRS, 2, 4, 64, 256)       # p f v rr w
        sh = np.ascontiguousarray(sh.transpose(2, 1, 3, 0, 4))
        in_maps.append({"x": sh, "cmat": cm})
    return in_maps


def _unshard_outputs(results):
    # y[j, v, e, p, w]; j = t*64 + q*32 + j'; s = (t, q, e); h = 32v + j'
    ys = np.stack([np.asarray(r["y"]) for r in results])  # [8,128,4,2,12,128]
    ys = ys.reshape(NCORES, 2, 2, 32, 4, 2, 4, 3, 128)
    #      dims: (k, t, q, j', v, e, b, c, w)
    ys = ys.transpose(1, 2, 5, 6, 7, 0, 4, 3, 8)
    #      -> (t, q, e, b, c, k, v, j', w)
    ys = np.ascontiguousarray(ys).reshape(8, 4, 3, NCORES, 128, 128)
    ys = ys.astype(np.float32)
    return tuple(ys[s] for s in range(8))


def run(video, **spmd_kwargs):
    nc = _get_nc()
    res = run_bass_kernel_spmd(
        nc, _shard_inputs(video), core_ids=list(range(NCORES)), **spmd_kwargs
    )
    return _unshard_outputs(res.results), res


def kernel(video):
    out, _ = run(video)
    return out


# revision 13
# speedup vs baseline: 1.0812x; 1.0812x over previous
"""Level-1 3D Haar DWT on video [4,3,16,256,256] f32 -> 8 subbands
[4,3,8,128,128], pywt convention (cA=(x0+x1)/sqrt2, cD=(x0-x1)/sqrt2 over
frames, height, width).

Distribution: pure data parallel over the 8 frame pairs (F=16 -> 8
independent pairs); core k processes video[:, :, 2k:2k+2] with zero
cross-core communication.

Host side: inputs are cast to f16 (rel-err budget 2e-2 >> f16's ~5e-4
quantization) and laid out per core as x[v, f, rr, p, w] so every DMA
run is contiguous; outputs come back f16 and are upcast. This halves
HBM traffic on both sides -> ~17.6us memory roofline per core.

Per-core pipeline (Bass/Tile), ragged chunks of CH (b,c) pairs, v in
0..3 indexing 64-row blocks of each frame:
  load (sync HWDGE):  X[v] = x[v, f, rr, p0:p0+CH, :]  [128=(f,rr), CH*256]
  F+H stage (PE): stationary C[128,128] (+-2^-1.5, 4 nonzeros/col)
    pairs frames (f) and adjacent rows (rr=2j'+r) in ONE matmul pass:
    out j = t*64+q*32+j'. The whole 1/sqrt8 scale is folded into C so
    the W stage is a plain tensor_tensor (no custom-DVE uop table in
    the preamble). A few warmup matmuls run during the preamble dead
    zone to bring the PE out of its low p-state. (Folding the W stage
    into PE as parity-accumulating matmuls doubles PE columns and was
    measured slower - the PE never ramps to full clock here.)
  evac: PSUM pair-tile -> SBUF f16. v-pair 0: ACT copies odd w cols
    only and DVE combines even(PSUM)+odd(SBUF) at 1x. v-pair 1: ACT
    copies both parities so DVE runs all-SBUF f16 at 2x. This splits
    the evac+combine work evenly (~11.8us each).
  W stage (DVE): tensor_tensor add/sub -> YU f16.
  store (gpsimd SWDGE, freeing the scalar ring for ACT): per v-pair,
    y[j, v, e, p, w], 1KB runs.

Output DRAM y[j, v, e, p, w]: subband s = (t, q, e) with j = t*64+
q*32+j', h = 32v + j'.
"""

import math

import numpy as np

import concourse.bacc as bacc
import concourse.mybir as mybir
from concourse.bass_utils import run_bass_kernel_spmd
from concourse.tile import TileContext

F16 = mybir.dt.float16
F32 = mybir.dt.float32
NCORES = 8
NPAIRS = 12
CHUNKS = (2, 4, 4, 2)   # ragged: short first/last chunks trim fill/drain
CHMAX = max(CHUNKS)
C3 = (1.0 / math.sqrt(2.0)) ** 3
NWARM = 10

_CACHE = {}


def _cmat():
    """C[i, o]: i = f*64 + 2j'+r, o = t*64 + q*32 + j'; entry
    C3*sF(t,f)*sH(q,r) with a=(+,+), d=(+,-)."""
    c = np.zeros((128, 128), np.float16)
    for t in range(2):
        for q in range(2):
            for jp in range(32):
                o = t * 64 + q * 32 + jp
                for f in range(2):
                    sf = -1.0 if (t == 1 and f == 1) else 1.0
                    for r in range(2):
                        sh = -1.0 if (q == 1 and r == 1) else 1.0
                        c[f * 64 + 2 * jp + r, o] = np.float16(C3) * sf * sh
    return c


def _build_bass():
    nc = bacc.Bacc()
    x = nc.dram_tensor("x", [4, 2, 64, NPAIRS, 256], F16, kind="ExternalInput")
    cm = nc.dram_tensor("cmat", [128, 128], F16, kind="ExternalInput")
    y = nc.dram_tensor("y", [128, 4, 2, NPAIRS, 128], F16,
                       kind="ExternalOutput")

    add = mybir.AluOpType.add
    sub = mybir.AluOpType.subtract

    with TileContext(nc) as tc:
        with tc.tile_pool(name="const", bufs=1) as cpool, \
             tc.tile_pool(name="io", bufs=3) as io_pool, \
             tc.tile_pool(name="mid", bufs=3) as mid_pool, \
             tc.tile_pool(name="ps", bufs=1, space="PSUM") as ps_pool:
            Ct = cpool.tile([128, 128], F16, name="Ct")
            # on sync, ahead of the X loads: scalar's hoisted
            # ACT_TABLE_LOAD would delay Ct (and the PE warmup) by ~1.5us
            nc.sync.dma_start(out=Ct[:, :], in_=cm[:, :])
            # PE p-state warmup in the preamble shadow (results unused;
            # the first real matmul below overwrites with start=True)
            Pw = ps_pool.tile([128, 2, CHMAX * 256], F32, name="Pw", tag="Q0")
            for _ in range(NWARM):
                nc.tensor.matmul(Pw[:, 0, 0:128], Ct[:, :], Ct[:, :])
            p0 = 0
            for ci, CH in enumerate(CHUNKS):
                # one ~0.5MB load per v-pair: big DMAs keep the HWDGE ring
                # at line rate, pair granularity keeps the first-MM dep fine
                Xp, Pp = [], []
                for t in range(2):
                    Xt = io_pool.tile([128, 2, CH * 256], F16, name="X",
                                      tag=f"X{t}", bufs=3,
                                      padded_shape=[128, 2, CHMAX * 256])
                    nc.sync.dma_start(
                        out=Xt[:, :, :],
                        in_=x[2 * t:2 * t + 2, :, :, p0:p0 + CH, :]
                            .rearrange("v f rr p w -> (f rr) v (p w)"),
                    )
                    Xp.append(Xt)
                YU = io_pool.tile([128, 4, 2, CH * 128], F16, name="YU",
                                  tag="YU",
                                  padded_shape=[128, 4, 2, CHMAX * 128])
                # all 8 MMs of the chunk back-to-back: longer PE bursts
                # ride the p-state up; evacs overlap on ACT/DVE
                for t in range(2):      # v-pair tiles: v = 2t, 2t+1
                    P2 = ps_pool.tile([128, 2, CH * 256], F32, name="P",
                                      tag=f"Q{t}",
                                      padded_shape=[128, 2, CHMAX * 256])
                    for dv in range(2):
                        for n0 in range(0, CH * 256, 512):  # 1 PSUM bank/mm
                            n1 = min(n0 + 512, CH * 256)
                            nc.tensor.matmul(P2[:, dv, n0:n1], Ct[:, :],
                                             Xp[t][:, dv, n0:n1])
                    Pp.append(P2)
                for t in range(2):
                    P2 = Pp[t]
                    Ps = P2.rearrange("j u (pw r) -> j u pw r", r=2)
                    Pe, Po = Ps[:, :, :, 0], Ps[:, :, :, 1]
                    # PSUM released by two short parallel copies (ACT odds,
                    # DVE evens); the TTs then run all-SBUF f16 at 2x and
                    # never touch PSUM -> fast PSUM recycle for the PE
                    Od = mid_pool.tile([128, 2, CH * 128], F16, name="Od",
                                       tag=f"O{t}",
                                       padded_shape=[128, 2, CHMAX * 128])
                    nc.scalar.copy(Od[:, :, :], Po)
                    Ev = mid_pool.tile([128, 2, CH * 128], F16, name="Ev",
                                       tag=f"E{t}",
                                       padded_shape=[128, 2, CHMAX * 128])
                    nc.vector.tensor_scalar_mul(Ev[:, :, :], Pe, 1.0)
                    nc.vector.tensor_tensor(
                        YU[:, 2 * t:2 * t + 2, 0, :], Ev[:, :, :],
                        Od[:, :, :], add)
                    nc.vector.tensor_tensor(
                        YU[:, 2 * t:2 * t + 2, 1, :], Ev[:, :, :],
                        Od[:, :, :], sub)
                    # store on the SAME sync ring as the loads: the HWDGE
                    # path starves the second ring while the first has
                    # queued work, so R/W must interleave in issue order
                    nc.sync.dma_start(
                        out=y[:, 2 * t:2 * t + 2, :, p0:p0 + CH, :]
                            .rearrange("j v e p w -> j v e (p w)"),
                        in_=YU[:, 2 * t:2 * t + 2, :, :],
                    )
                p0 += CH
    nc.compile()
    return nc


def _get_nc():
    if "nc" not in _CACHE:
        _CACHE["nc"] = _build_bass()
    return _CACHE["nc"]


def _shard_inputs(video):
    video = np.asarray(video, dtype=np.float16)
    cm = _cmat()
    in_maps = []
    for k in range(NCORES):
        sh = video[:, :, 2 * k:2 * k + 2]            # [4,3,2,256,256]
        sh = sh.reshape(NPAIRS, 2, 4, 64, 256)       # p f v rr w
        sh = np.ascontiguousarray(sh.transpose(2, 1, 3, 0, 4))
        in_maps.append({"x": sh, "cmat": cm})
    return in_maps


def _unshard_outputs(results):
    # y[j, v, e, p, w]; j = t*64 + q*32 + j'; s = (t, q, e); h = 32v + j'
    ys = np.stack([np.asarray(r["y"]) for r in results])  # [8,128,4,2,12,128]
    ys = ys.reshape(NCORES, 2, 2, 32, 4, 2, 4, 3, 128)
    #      dims: (k, t, q, j', v, e, b, c, w)
    ys = ys.transpose(1, 2, 5, 6, 7, 0, 4, 3, 8)
    #      -> (t, q, e, b, c, k, v, j', w)
    ys = np.ascontiguousarray(ys).reshape(8, 4, 3, NCORES, 128, 128)
    ys = ys.astype(np.float32)
    return tuple(ys[s] for s in range(8))


def run(video, **spmd_kwargs):
    nc = _get_nc()
    res = run_bass_kernel_spmd(
        nc, _shard_inputs(video), core_ids=list(range(NCORES)), **spmd_kwargs
    )
    return _unshard_outputs(res.results), res


def kernel(video):
    out, _ = run(video)
    return out
